# revision 1
# baseline (speedup 1.0000x reference)
"""Bass/Trainium2 kernel for nn_NodesToEdges (gnn_message_passing).

out[b,i,j,:] = rms(edges[b,i,j,:])*g_e @ We + rms(nodes[b,i,:])*g_n @ Wr
             + rms(nodes[b,j,:])*g_n @ Wc + bias

Strategy: shard over i (rows) across 8 cores; tiny node path on host.
The edge path is memory-bound, so the device moves bf16: the host
converts edges to bf16 and upconverts the bf16 result (error ~0.5%,
well under the 2e-2 gate), halving HBM traffic vs f32.

Device schedule, per SLOT of 4 blocks = 2 pairs, [128 part=(a,p),
1024 free=(s,r,e)] where s = pair-within-slot, j = 8p+r:

  in-DMA (SP) -> square (Pool, bf16) -> group-reduce to [128,16] (DVE)
  -> sqrt(mean+eps) (ACT) -> reciprocal (DVE) -> prescale x*inv (Pool)
  -> PE transpose 8 chunks (bf16, into one PSUM bank) -> xT (ACT copy)
  -> per pair: col_proj matmul (const selp/cpmov operands) + 4 fused
     matmul-transposes (xT chunk stationary, blockdiag-Wg moving) give
     row-major y + col_proj + bias in PSUM
  -> final add of row_proj via free-dim-broadcast operand (DVE)
  -> out-DMA (SP).

All matmuls are bf16 (1 cycle/row on PE, f32 PSUM accumulate). The
row_proj operand rpexp[(a,p), 64t+e'] duplicates row_proj across the
64 p-partitions since DVE cannot broadcast across partitions.
"""

import numpy as np

B, N, DE, DN = 2, 512, 64, 128
NCORES = 8
IPC = N // NCORES          # 64 i-rows per core
NBLK = B * IPC             # 128 blocks of 512 rows per core
NPAIR = NBLK // 2          # 64 pairs
NV = NPAIR // 2            # 32 slots of 2 pairs
EPS = float(np.finfo(np.float32).eps)


def _build_nc(nv=NV):
    from contextlib import ExitStack

    import concourse.bass as bass
    import concourse.mybir as mybir

    f32 = mybir.dt.float32
    bf16 = mybir.dt.bfloat16
    SQRT = mybir.ActivationFunctionType.Sqrt

    nc = bass.Bass()
    npair = 2 * nv
    nblk = 2 * npair
    x_d = nc.declare_dram_parameter("x", [nblk, N, DE], bf16, isOutput=False)
    cpmov_d = nc.declare_dram_parameter("cpmov", [128, 1024], bf16, isOutput=False)
    selp_d = nc.declare_dram_parameter("selp", [128, 128], bf16, isOutput=False)
    rpexp_d = nc.declare_dram_parameter("rpexp", [128, NPAIR * DE], bf16, isOutput=False)
    wgblk_d = nc.declare_dram_parameter("wgblk", [128, 128], bf16, isOutput=False)
    id128_d = nc.declare_dram_parameter("id128", [128, 128], bf16, isOutput=False)
    out_d = nc.declare_dram_parameter("out", [nblk, N, DE], bf16, isOutput=True)

    # buffer depths (slot granularity unless noted)
    DX2 = 6    # xin slots
    DQ2 = 3    # sq
    DS2 = 3    # ss / rms / inv
    DXS2 = 3   # xs
    DXT2 = 3   # xT
    DO2 = 8    # outsb
    DP1B = 3   # ps1 psum banks (bf16, 1 bank each)
    DPE2 = 2   # psE psum tensors (f32, [128,1024] = 2 banks, per SLOT)
    NSO = 8    # out-completion sem ring (= DO2)
    CSPLIT = 832  # psE-drain column split: ACT [0:832], DVE [832:1024]
    OUTLAG = 8  # out2(v) issued at SP iter v+OUTLAG

    st = ExitStack()
    with st:
        sb = lambda shape, dt, name: st.enter_context(
            nc.sbuf_tensor(name, shape, dt)
        )
        cpmov = sb([128, 1024], bf16, "cpmov_sb")
        selp = sb([128, 128], bf16, "selp_sb")
        rpexp = sb([128, NPAIR * DE], bf16, "rpexp_sb")
        wgblk = sb([128, 128], bf16, "wgblk_sb")
        id128 = sb([128, 128], bf16, "id128_sb")
        epsb = sb([128, 1], f32, "epsb")
        xin = [sb([128, 1024], bf16, f"xin{i}") for i in range(DX2)]
        sq = [sb([128, 1024], bf16, f"sq{i}") for i in range(DQ2)]
        ss = [sb([128, 16], f32, f"ss{i}") for i in range(DS2)]
        rms = [sb([128, 16], f32, f"rms{i}") for i in range(DS2)]
        inv = [sb([128, 16], f32, f"inv{i}") for i in range(DS2)]
        xs = [sb([128, 1024], bf16, f"xs{i}") for i in range(DXS2)]
        xT = [sb([128, 1024], bf16, f"xT{i}") for i in range(DXT2)]
        outsb = [sb([128, 1024], bf16, f"outsb{i}") for i in range(DO2)]
        ps1 = [
            st.enter_context(nc.psum_tensor(f"ps1{i}", [128, 1024], bf16))
            for i in range(DP1B)
        ]
        psE = [
            st.enter_context(nc.psum_tensor(f"psE{i}", [128, 1024], f32))
            for i in range(DPE2)
        ]

        sem = lambda name: st.enter_context(nc.semaphore(name))
        s_c = sem("s_c")
        s_in = [sem(f"s_in{i}") for i in range(DX2)]
        s_out = [sem(f"s_out{i}") for i in range(NSO)]
        s_pl = sem("s_pl")
        s_dve = sem("s_dve")
        s_act = sem("s_act")
        s_pe = sem("s_pe")

        # stages (slot-iteration): in2(v)@SP v, square2(v)@Pool v,
        # reduce2(v)@DVE v, sqrt2(v)@ACT v, recip2(v)@DVE v+1,
        # prescale2(v)@Pool v+1, fwdT2(v)@PE v+2, copy12(v)@ACT v+3,
        # mm(2v), mm(2v+1)@PE v+3 (after copy12 lands mid-iter; PE has
        # ~50% slack), copy2(v)@ACT v+4 (psE f32 -> outsb bf16),
        # rpadd(v)@DVE v+5 (in-place, all-bf16 so DVE runs in 2x mode),
        # out2(v)@SP v+OUTLAG
        done = {}
        for v in range(nv):
            done[("in2", v)] = 16 * (v // DX2 + 1)    # on s_in[v % DX2]
            done[("out2", v)] = 16 * (v // NSO + 1)   # on s_out[v % NSO]
        # s_pl: Pool order per iter = square2(v), prescale2(v-1)
        c = 0
        for v in range(nv + 1):
            if v < nv:
                c += 1; done[("square2", v)] = c
            if 1 <= v < nv + 1:
                c += 1; done[("prescale2", v - 1)] = c
        # s_dve: recip2(v-1), reduce2(v), copy2b(v-4), rpadd(v-5)
        c = 0
        for v in range(nv + 5):
            if 1 <= v < nv + 1:
                c += 1; done[("recip2", v - 1)] = c
            if v < nv:
                c += 1; done[("reduce2", v)] = c
            if 4 <= v < nv + 4:
                c += 1; done[("copy2b", v - 4)] = c
            if 5 <= v < nv + 5:
                c += 1; done[("rpadd", v - 5)] = c
        # s_act: copy12(v-3), copy2a(v-4), sqrt2(v)
        c = 0
        for v in range(nv + 4):
            if 3 <= v < nv + 3:
                c += 1; done[("copy12", v - 3)] = c
            if 4 <= v < nv + 4:
                c += 1; done[("copy2a", v - 4)] = c
            if v < nv:
                c += 1; done[("sqrt2", v)] = c
        # s_pe: fwdT2(v-2), mm(2(v-3)), mm(2(v-3)+1)
        c = 0
        for v in range(nv + 3):
            if 2 <= v < nv + 2:
                c += 1; done[("fwdT2", v - 2)] = c
            if 3 <= v < nv + 3:
                c += 1; done[("mm", 2 * (v - 3))] = c
                c += 1; done[("mm", 2 * (v - 3) + 1)] = c

        CONST_TARGET = 5 * 16

        def slot_ap(tens, v):
            # [(a p), s, (r e)]: s kept as its own dim (its stride jumps
            # over the a-dim, so it cannot merge with (r e))
            return (
                tens[4 * v : 4 * v + 4]
                .rearrange("(s a) (p r) e -> s a p r e", s=2, p=64)
                .rearrange("s a p r e -> a p s r e")
                .rearrange("a p s r e -> (a p) s (r e)")
            )

        def sbuf_slot3(t):
            return t[:].rearrange("P (s f) -> P s f", s=2)

        def in_src(v):
            return slot_ap(x_d, v)

        def out_dst(v):
            return slot_ap(out_d, v)

        with nc.Block() as block:

            @block.sync
            def _(sync):
                consts_emitted = False
                for v in range(nv + OUTLAG):
                    # two xin loads lead; consts follow (nothing needs
                    # them until PE iter 2 / DVE's first final)
                    if v == 2 or (v >= nv and not consts_emitted):
                        consts_emitted = True
                        for cdst, csrc in (
                            (cpmov, cpmov_d[:]),
                            (selp, selp_d[:]),
                            (rpexp, rpexp_d[:]),
                            (wgblk, wgblk_d[:]),
                            (id128, id128_d[:]),
                        ):
                            sync.dma_start(out=cdst[:], in_=csrc).then_inc(
                                s_c, 16
                            )
                    # in2(v)
                    if v < nv:
                        if v >= DX2:
                            sync.wait_ge(s_pl, done[("prescale2", v - DX2)])
                        sync.dma_start(
                            out=sbuf_slot3(xin[v % DX2]), in_=in_src(v)
                        ).then_inc(s_in[v % DX2], 16)
                    # out2(v-OUTLAG): trails rpadd() by 3 DVE iterations
                    # so this wait never throttles the in() prefetch
                    if OUTLAG <= v < nv + OUTLAG:
                        w = v - OUTLAG
                        sync.wait_ge(s_dve, done[("rpadd", w)])
                        sync.dma_start(
                            out=out_dst(w), in_=sbuf_slot3(outsb[w % DO2])
                        ).then_inc(s_out[w % NSO], 16)

            @block.gpsimd
            def _(pool):
                for v in range(nv + 1):
                    # square2(v)
                    if v < nv:
                        pool.wait_ge(s_in[v % DX2], done[("in2", v)])
                        if v >= DQ2:
                            pool.wait_ge(s_dve, done[("reduce2", v - DQ2)])
                        nc.gpsimd.tensor_mul(
                            sq[v % DQ2][:], xin[v % DX2][:], xin[v % DX2][:]
                        ).then_inc(s_pl, 1)
                    # prescale2(v-1)
                    if 1 <= v < nv + 1:
                        w = v - 1
                        pool.wait_ge(s_dve, done[("recip2", w)])
                        if w >= DXS2:
                            pool.wait_ge(s_pe, done[("fwdT2", w - DXS2)])
                        nc.gpsimd.tensor_mul(
                            xs[w % DXS2][:].rearrange("p (g e) -> p g e", e=DE),
                            xin[w % DX2][:].rearrange("p (g e) -> p g e", e=DE),
                            inv[w % DS2][:].unsqueeze(-1).broadcast_to([128, 16, DE]),
                        ).then_inc(s_pl, 1)

            @block.vector
            def _(vector):
                nc.vector.memset(epsb[:], EPS)
                for v in range(nv + 5):
                    # recip2(v-1)
                    if 1 <= v < nv + 1:
                        w = v - 1
                        vector.wait_ge(s_act, done[("sqrt2", w)])
                        if w >= DS2:
                            vector.wait_ge(s_pl, done[("prescale2", w - DS2)])
                        nc.vector.reciprocal(
                            inv[w % DS2][:], rms[w % DS2][:]
                        ).then_inc(s_dve, 1)
                    # reduce2(v)
                    if v < nv:
                        vector.wait_ge(s_pl, done[("square2", v)])
                        if v >= DS2:
                            vector.wait_ge(s_act, done[("sqrt2", v - DS2)])
                        nc.vector.tensor_reduce(
                            ss[v % DS2][:],
                            sq[v % DQ2][:].rearrange("p (g e) -> p g e", e=DE),
                            axis=mybir.AxisListType.X,
                            op=mybir.AluOpType.add,
                        ).then_inc(s_dve, 1)
                    # copy2b(v-4): DVE's share of the psE drain
                    # (columns CSPLIT..1024)
                    if 4 <= v < nv + 4:
                        w = v - 4
                        vector.wait_ge(s_pe, done[("mm", 2 * w + 1)])
                        if w >= DO2:
                            vector.wait_ge(
                                s_out[(w - DO2) % NSO],
                                done[("out2", w - DO2)],
                            )
                        nc.vector.tensor_copy(
                            outsb[w % DO2][:, CSPLIT:],
                            psE[w % DPE2][:, CSPLIT:],
                        ).then_inc(s_dve, 1)
                    # rpadd(v-5): in-place row_proj add on the bf16
                    # outsb slot (all operands 2-byte SBUF -> 2x DVE mode)
                    if 5 <= v < nv + 5:
                        w = v - 5
                        if w == 0:
                            vector.wait_ge(s_c, CONST_TARGET)
                        vector.wait_ge(s_act, done[("copy2a", w)])
                        vector.wait_ge(s_dve, done[("copy2b", w)])
                        nc.vector.tensor_add(
                            outsb[w % DO2][:]
                            .rearrange("p (s g e) -> p s g e", s=2, e=DE),
                            outsb[w % DO2][:]
                            .rearrange("p (s g e) -> p s g e", s=2, e=DE),
                            rpexp[:, 2 * DE * w : 2 * DE * (w + 1)]
                            .rearrange("p (s e) -> p s e", s=2)
                            .unsqueeze(2)
                            .broadcast_to([128, 2, 8, DE]),
                        ).then_inc(s_dve, 1)

            @block.scalar
            def _(scalar):
                for v in range(nv + 4):
                    # copy12(v-3): ps1 -> xT
                    if 3 <= v < nv + 3:
                        w = v - 3
                        scalar.wait_ge(s_pe, done[("fwdT2", w)])
                        if w >= DXT2:
                            scalar.wait_ge(
                                s_pe, done[("mm", 2 * (w - DXT2) + 1)]
                            )
                        nc.scalar.copy(xT[w % DXT2][:], ps1[w % DP1B][:]).then_inc(
                            s_act, 1
                        )
                    # copy2a(v-4): ACT's share of the psE drain
                    # (columns 0..CSPLIT)
                    if 4 <= v < nv + 4:
                        w = v - 4
                        scalar.wait_ge(s_pe, done[("mm", 2 * w + 1)])
                        if w >= DO2:
                            scalar.wait_ge(
                                s_out[(w - DO2) % NSO],
                                done[("out2", w - DO2)],
                            )
                        nc.scalar.copy(
                            outsb[w % DO2][:, :CSPLIT], psE[w % DPE2][:, :CSPLIT]
                        ).then_inc(s_act, 1)
                    # sqrt2(v)
                    if v < nv:
                        scalar.wait_ge(s_dve, done[("reduce2", v)])
                        if v >= DS2:
                            scalar.wait_ge(s_dve, done[("recip2", v - DS2)])
                        nc.scalar.activation(
                            rms[v % DS2][:], ss[v % DS2][:], SQRT,
                            bias=epsb[:], scale=1.0 / DE,
                        ).then_inc(s_act, 1)

            @block.tensor
            def _(tensor):
                tensor.wait_ge(s_c, CONST_TARGET)
                for v in range(nv + 3):
                    # fwdT2(v-2): 8 chunk transposes into one bf16 bank
                    if 2 <= v < nv + 2:
                        w = v - 2
                        tensor.wait_ge(s_pl, done[("prescale2", w)])
                        if w >= DP1B:
                            tensor.wait_ge(s_act, done[("copy12", w - DP1B)])
                        for q in range(8):
                            mm = nc.tensor.transpose(
                                ps1[w % DP1B][:, 128 * q : 128 * q + 128],
                                xs[w % DXS2][:, 128 * q : 128 * q + 128],
                                id128[:],
                            )
                            if q == 7:
                                mm.then_inc(s_pe, 1)
                    # mm(2(v-3)+h): col_proj matmul + 4 fused
                    # matmul-transposes -> row-major y in psE
                    if 3 <= v < nv + 3:
                        w = v - 3
                        tensor.wait_ge(s_act, done[("copy12", w)])
                        if w >= DPE2:
                            tensor.wait_ge(s_act, done[("copy2a", w - DPE2)])
                            tensor.wait_ge(s_dve, done[("copy2b", w - DPE2)])
                        for h in (0, 1):
                            t = 2 * w + h
                            bslice = 512 * ((2 * t) // IPC)
                            nc.tensor.matmul(
                                psE[w % DPE2][:, 512 * h : 512 * h + 512],
                                selp[:],
                                cpmov[:, bslice : bslice + 512],
                                start=True, stop=False,
                                skip_group_check=True,
                            )
                            for q in range(4):
                                mm = nc.tensor.matmul(
                                    psE[w % DPE2][
                                        :, 512 * h + 128 * q : 512 * h + 128 * q + 128
                                    ],
                                    xT[w % DXT2][
                                        :, 512 * h + 128 * q : 512 * h + 128 * q + 128
                                    ],
                                    wgblk[:],
                                    start=False, stop=True,
                                    skip_group_check=True,
                                )
                                if q == 3:
                                    mm.then_inc(s_pe, 1)

    return nc


_NC_CACHE = {}


def _get_nc():
    if "nc" not in _NC_CACHE:
        _NC_CACHE["nc"] = _build_nc()
    return _NC_CACHE["nc"]


def _make_in_maps(edges, nodes, g_node, g_edge, W, b):
    import ml_dtypes

    bf16 = ml_dtypes.bfloat16
    edges = np.ascontiguousarray(edges, dtype=np.float32)
    nodes = np.ascontiguousarray(nodes, dtype=np.float32)
    g_node = np.asarray(g_node, dtype=np.float32)
    g_edge = np.asarray(g_edge, dtype=np.float32)
    W = np.asarray(W, dtype=np.float32)
    b = np.asarray(b, dtype=np.float32)

    # ---- host: tiny node path (B*N*dn = 131K elems)
    ms = np.mean(np.square(nodes), axis=-1, keepdims=True)
    nodes_n = nodes / np.sqrt(ms + EPS) * g_node  # [B, N, 128]
    Wr, Wc, We = W[:DN], W[DN : 2 * DN], W[2 * DN :]
    row_proj = (nodes_n @ Wr).astype(np.float32)  # [B, N, 64]
    col_proj = (nodes_n @ Wc).astype(np.float32)  # [B, N, 64]
    Wg = (g_edge[:, None] * We).astype(np.float32)  # fold g_edge into We

    # cpmov[k, 512b + (r,e)] = col_proj[b, 8k+r, e] + bias  (k < 64)
    cp = (col_proj + b).astype(np.float32).reshape(B, 64, 8 * DE)  # [2, 64, 512]
    cpmov = np.zeros((128, 1024), dtype=bf16)
    cpmov[:64, :512] = cp[0]
    cpmov[:64, 512:] = cp[1]
    # selp[k, (a,p)] = 1 iff k == p (k < 64)
    selp = np.zeros((128, 128), dtype=bf16)
    krow = np.arange(128)[:, None]
    selp[(krow < 64) & ((np.arange(128) % 64)[None, :] == krow)] = 1

    # block-diagonal Wg over the 2-row subindex
    wgblk = np.zeros((128, 128), dtype=bf16)
    wgblk[:64, :64] = Wg
    wgblk[64:, 64:] = Wg
    id128 = np.eye(128, dtype=bf16)

    in_maps = []
    for c in range(NCORES):
        xs = edges[:, c * IPC : (c + 1) * IPC]  # [B, 64, 512, 64]
        xs = np.ascontiguousarray(xs).reshape(NBLK, N, DE).astype(bf16)
        # rpexp[a*64+p, 64t+e] = row_proj[block 2t+a][e]  (p-duplicated)
        rp = row_proj[:, c * IPC : (c + 1) * IPC].reshape(NBLK, DE)  # [128, 64]
        rpexp = np.empty((128, NPAIR * DE), dtype=bf16)
        rpexp[:64, :] = rp[0::2].reshape(1, -1)
        rpexp[64:, :] = rp[1::2].reshape(1, -1)
        in_maps.append(
            {
                "x": xs,
                "cpmov": cpmov,
                "selp": selp,
                "rpexp": rpexp,
                "wgblk": wgblk,
                "id128": id128,
            }
        )
    return in_maps


def kernel(edges, nodes, g_node, g_edge, W, b):
    in_maps = _make_in_maps(edges, nodes, g_node, g_edge, W, b)

    from concourse.bass_utils import run_bass_kernel_spmd

    nc = _get_nc()
    res = run_bass_kernel_spmd(nc, in_maps, list(range(NCORES)))

    out = np.empty((B, N, N, DE), dtype=np.float32)
    for c in range(NCORES):
        oc = res.results[c]["out"].astype(np.float32).reshape(B, IPC, N, DE)
        out[:, c * IPC : (c + 1) * IPC] = oc
    return out


if __name__ == "__main__":
    rng = np.random.default_rng(0)
    edges = rng.standard_normal((B, N, N, DE), dtype=np.float32)
    nodes = rng.standard_normal((B, N, DN), dtype=np.float32)
    g_node = np.ones(DN, np.float32)
    g_edge = np.ones(DE, np.float32)
    W = rng.standard_normal((2 * DN + DE, DE), dtype=np.float32) / 18.0
    b = (rng.standard_normal(DE) * 0.01).astype(np.float32)
    o = kernel(edges, nodes, g_node, g_edge, W, b)
    print(o.shape, o.dtype)



# revision 3
# speedup vs baseline: 1.8004x; 1.8004x over previous
"""Bass/Trainium2 kernel for nn_NodesToEdges (gnn_message_passing).

out[b,i,j,:] = rms(edges[b,i,j,:])*g_e @ We + rms(nodes[b,i,:])*g_n @ Wr
             + rms(nodes[b,j,:])*g_n @ Wc + bias

Strategy: shard over i (rows) across 8 cores. The device computes ONLY
the heavy edge term y = rms(edges)*g_e @ We in bf16; the tiny node
projections + bias are added on the host in f32 (broadcast adds).

Device schedule, per SLOT of 4 blocks, working set [128 part, 1024]:

  xT loaded TRANSPOSED straight from DRAM via xbar transpose-DMAs
  (host pre-arranges x as [slot, (q a p), (r2 e)]; split SP 960 rows /
  ACT 64 rows); DVE squares xT (bf16 2x); PE reduces sumsq via 8
  tiny-moving matmuls (stat=sqT chunk, mov=2-col selector) ->
  psS[128,16]; ACT sqrt(mean+eps); DVE recip -> inv[128,16]; PE
  y-matmuls (stat=raw xT chunk, mov=blockdiag Wg) -> psE row-major
  f32; drain psE -> ysb bf16 split ACT/DVE by columns; Pool scales
  ysb in place by inv (per-edge rsqrt, SBUF-only so GPSIMD is legal);
  out-DMA of ysb rotates across Pool/SP/ACT queues (2/2/1 of 5).

Stage lags (iteration = slot + L): inT@0 square@1 {sumsq,y}@2
{sqrt,drain}@3 recip@4 scale@5 out@6.
"""

import numpy as np

B, N, DE, DN = 2, 512, 64, 128
NCORES = 8
IPC = N // NCORES          # 64 i-rows per core
NBLK = B * IPC             # 128 blocks of 512 rows per core
NV = NBLK // 4             # 32 slots of 4 blocks
EPS = float(np.finfo(np.float32).eps)

# tunables
DR_A = 192                 # drain: ACT raw [0:DR_A], DVE fused-scale rest
GR_A = DR_A // DE          # groups in the ACT share
XPAD = 64                  # xbar tail-tile guard rows (DMA sem can
                           # fire before the last ~4 tiles land)
DXT = 6                    # xT buffers
DSQ = 3
DRMS = 3
DINV = 4
DPE = 3                    # psE buffers (2 banks each)
DY = 10                    # ysb2 buffers (= out-queue rotation period)
DYR = 3                    # raw ysb buffers (ACT share only)
LOUT = 9                   # out(v) at iteration v+LOUT
# out queue by v%10: 2 sp, 7 act, 1 pool
OUTQ = ["sp", "act", "act", "act", "pool",
        "act", "sp", "act", "act", "act"]


def _build_nc(nv=NV):
    from contextlib import ExitStack

    import concourse.bass as bass
    import concourse.mybir as mybir

    f32 = mybir.dt.float32
    bf16 = mybir.dt.bfloat16
    SQRT = mybir.ActivationFunctionType.Sqrt

    nc = bass.Bass()
    nblk = 4 * nv
    x_d = nc.declare_dram_parameter("x", [nv, 1024 + XPAD, 128], bf16, isOutput=False)
    wgblk_d = nc.declare_dram_parameter("wgblk", [128, 128], bf16, isOutput=False)
    sel2_d = nc.declare_dram_parameter("sel2", [128, 2], bf16, isOutput=False)
    out_d = nc.declare_dram_parameter("out", [nblk, N, DE], bf16, isOutput=True)

    st = ExitStack()
    with st:
        sb = lambda shape, dt, name: st.enter_context(
            nc.sbuf_tensor(name, shape, dt)
        )
        wgblk = sb([128, 128], bf16, "wgblk_sb")
        sel2 = sb([128, 2], bf16, "sel2_sb")
        epsb = sb([128, 1], f32, "epsb")
        dum = sb([128, 1], f32, "dum")
        xT = [sb([128, 1024 + XPAD], bf16, f"xT{i}") for i in range(DXT)]
        sqT = [sb([128, 1024], bf16, f"sqT{i}") for i in range(DSQ)]
        rms = [sb([128, 16], f32, f"rms{i}") for i in range(DRMS)]
        inv = [sb([128, 16], f32, f"inv{i}") for i in range(DINV)]
        ysb = [sb([128, DR_A], bf16, f"ysb{i}") for i in range(DYR)]
        ysb2 = [sb([128, 1024], bf16, f"ysb2_{i}") for i in range(DY)]
        psS = [
            st.enter_context(nc.psum_tensor(f"psS{i}", [128, 512], f32))
            for i in range(2)
        ]
        psE = [
            st.enter_context(nc.psum_tensor(f"psE{i}", [128, 1024], f32))
            for i in range(DPE)
        ]

        sem = lambda name: st.enter_context(nc.semaphore(name))
        s_c = sem("s_c")
        s_eps = sem("s_eps")
        s_in = [sem(f"s_in{i}") for i in range(DXT)]
        s_sq = sem("s_sq")
        s_rms = sem("s_rms")
        s_inv = sem("s_inv")
        s_y = sem("s_y")
        s_dra = sem("s_dra")
        s_drd = sem("s_drd")
        s_sc = sem("s_sc")
        s_o = [sem(f"s_o{i}") for i in range(DY)]

        def out_ap(v):
            return (
                out_d[4 * v : 4 * v + 4]
                .rearrange("(s a) (p r) e -> s a p r e", s=2, p=64)
                .rearrange("s a p r e -> a p s r e")
                .rearrange("a p s r e -> (a p) s (r e)")
            )

        def emit_out(eng, w):
            eng.wait_ge(s_sc, w + 1)
            eng.wait_ge(s_drd, w + 1)
            eng.dma_start(
                out=out_ap(w),
                in_=ysb2[w % DY][:].rearrange("P (s f) -> P s f", s=2),
            ).then_inc(s_o[w % DY], 16)

        def wait_ysb2_free(eng, v):
            # ysb2[v % DY] was last read by out(v - DY)
            if v >= DY:
                w = v - DY
                eng.wait_ge(s_o[w % DY], 16 * (w // DY + 1))

        with nc.Block() as block:

            @block.sync
            def _(sync):
                for t in range(nv + LOUT):
                    # inT(t): whole slot (+ guard rows) in one xbar DMA
                    if t < nv:
                        if t >= DXT:
                            sync.wait_ge(s_y, t - DXT + 1)
                            sync.wait_ge(s_sq, t - DXT + 1)
                        sync.dma_start_transpose(
                            xT[t % DXT][:], x_d[t]
                        ).then_inc(s_in[t % DXT], 16)
                    if LOUT <= t < nv + LOUT and OUTQ[(t - LOUT) % DY] == "sp":
                        emit_out(sync, t - LOUT)

            @block.scalar
            def _(scalar):
                for cdst, csrc in ((wgblk, wgblk_d[:]), (sel2, sel2_d[:])):
                    scalar.dma_start(out=cdst[:], in_=csrc).then_inc(s_c, 16)
                # dummy Sqrt primes the ACT function table off the hot path
                scalar.wait_ge(s_c, 32)
                nc.scalar.activation(
                    dum[:], wgblk[:, :1], SQRT, bias=0.0, scale=0.0
                )

                for t in range(nv + LOUT):
                    # sqrt(t-5)
                    if 5 <= t < nv + 5:
                        w = t - 5
                        if w == 0:
                            scalar.wait_ge(s_eps, 1)
                        scalar.wait_ge(s_y, w + 1)
                        if w >= DRMS:
                            scalar.wait_ge(s_inv, w - DRMS + 1)
                        nc.scalar.activation(
                            rms[w % DRMS][:], psS[w % 2][:, :16], SQRT,
                            bias=epsb[:], scale=1.0 / DE,
                        ).then_inc(s_rms, 1)
                    # drain_a(t-6): cols [0:DR_A], raw psE -> ysb bf16
                    if 6 <= t < nv + 6:
                        w = t - 6
                        scalar.wait_ge(s_y, w + 1)
                        if w >= DYR:
                            scalar.wait_ge(s_sc, w - DYR + 1)
                        nc.scalar.copy(
                            ysb[w % DYR][:], psE[w % DPE][:, :DR_A]
                        ).then_inc(s_dra, 1)
                    if LOUT <= t < nv + LOUT and OUTQ[(t - LOUT) % DY] == "act":
                        emit_out(scalar, t - LOUT)

            @block.vector
            def _(vector):
                nc.vector.memset(epsb[:], EPS).then_inc(s_eps, 1)
                for t in range(nv + LOUT):
                    # recip(t-6)
                    if 6 <= t < nv + 6:
                        w = t - 6
                        vector.wait_ge(s_rms, w + 1)
                        if w >= DINV:
                            vector.wait_ge(s_sc, w - DINV + 1)
                            vector.wait_ge(s_drd, w - DINV + 1)
                        nc.vector.reciprocal(
                            inv[w % DINV][:], rms[w % DRMS][:]
                        ).then_inc(s_inv, 1)
                    # drain_d(t-7): fused drain+scale, cols [DR_A:1024]
                    if 7 <= t < nv + 7:
                        w = t - 7
                        vector.wait_ge(s_y, w + 1)
                        vector.wait_ge(s_inv, w + 1)
                        wait_ysb2_free(vector, w)
                        nc.vector.tensor_mul(
                            ysb2[w % DY][:, DR_A:]
                            .rearrange("p (g e) -> p g e", e=DE),
                            psE[w % DPE][:, DR_A:]
                            .rearrange("p (g e) -> p g e", e=DE),
                            inv[w % DINV][:, GR_A:]
                            .unsqueeze(-1)
                            .broadcast_to([128, 16 - GR_A, DE]),
                        ).then_inc(s_drd, 1)

            @block.gpsimd
            def _(pool):
                for t in range(nv + LOUT):
                    # square(t-3) on Pool (DVE self-operand mul is unsafe
                    # on HW; Pool's is baseline-proven)
                    if 3 <= t < nv + 3:
                        w = t - 3
                        pool.wait_ge(s_in[w % DXT], 16 * (w // DXT + 1))
                        if w >= DSQ:
                            pool.wait_ge(s_y, w - DSQ + 1)
                        nc.gpsimd.tensor_mul(
                            sqT[w % DSQ][:],
                            xT[w % DXT][:, :1024],
                            xT[w % DXT][:, :1024],
                        ).then_inc(s_sq, 1)
                    # scale_p(t-7): ysb2[:, :DR_A] = ysb * inv (not in-place)
                    if 7 <= t < nv + 7:
                        w = t - 7
                        pool.wait_ge(s_inv, w + 1)
                        pool.wait_ge(s_dra, w + 1)
                        wait_ysb2_free(pool, w)
                        nc.gpsimd.tensor_mul(
                            ysb2[w % DY][:, :DR_A]
                            .rearrange("p (g e) -> p g e", e=DE),
                            ysb[w % DYR][:]
                            .rearrange("p (g e) -> p g e", e=DE),
                            inv[w % DINV][:, :GR_A]
                            .unsqueeze(-1)
                            .broadcast_to([128, GR_A, DE]),
                        ).then_inc(s_sc, 1)
                    if LOUT <= t < nv + LOUT and OUTQ[(t - LOUT) % DY] == "pool":
                        emit_out(pool, t - LOUT)

            @block.tensor
            def _(tensor):
                tensor.wait_ge(s_c, 32)
                for t in range(nv + 4):
                    if 4 <= t < nv + 4:
                        w = t - 4
                        # sumsq: 8 tiny matmuls -> psS[:, 2q:2q+2]
                        tensor.wait_ge(s_sq, w + 1)
                        if w >= 2:
                            tensor.wait_ge(s_rms, w - 1)
                        for q in range(8):
                            nc.tensor.matmul(
                                psS[w % 2][:, 2 * q : 2 * q + 2],
                                sqT[w % DSQ][:, 128 * q : 128 * q + 128],
                                sel2[:],
                                start=(q == 0), stop=(q == 7),
                                skip_group_check=True,
                            )
                        # y: 8 matmuls raw xT vs blockdiag Wg
                        if w >= DPE:
                            tensor.wait_ge(s_dra, w - DPE + 1)
                            tensor.wait_ge(s_drd, w - DPE + 1)
                        # (xT also read by Pool square; inT waits s_sq too)
                        for q in range(8):
                            mm = nc.tensor.matmul(
                                psE[w % DPE][:, 128 * q : 128 * q + 128],
                                xT[w % DXT][:, 128 * q : 128 * q + 128],
                                wgblk[:],
                                start=(q % 4 == 0), stop=(q % 4 == 3),
                                skip_group_check=True,
                            )
                            if q == 7:
                                mm.then_inc(s_y, 1)

    return nc


_NC_CACHE = {}


def _get_nc():
    if "nc" not in _NC_CACHE:
        _NC_CACHE["nc"] = _build_nc()
    return _NC_CACHE["nc"]


def _make_in_maps(edges, g_edge, We):
    import ml_dtypes

    bf16 = ml_dtypes.bfloat16
    Wg = (np.asarray(g_edge, dtype=np.float32)[:, None] * We).astype(np.float32)

    wgblk = np.zeros((128, 128), dtype=bf16)
    wgblk[:64, :64] = Wg.astype(bf16)
    wgblk[64:, 64:] = Wg.astype(bf16)
    sel2 = np.zeros((128, 2), dtype=bf16)
    sel2[:64, 0] = 1
    sel2[64:, 1] = 1

    in_maps = []
    for c in range(NCORES):
        # x device layout: [nv, 1024=(q:(s,rp), a, p), 128=(r2, e)]
        xs = edges[:, c * IPC : (c + 1) * IPC]  # [B, 64, 512, 64]
        xs = np.ascontiguousarray(xs).reshape(NBLK, N, DE)
        x7 = xs.reshape(NV, 2, 2, 64, 4, 2, 64)  # v s a p rp r2 e
        xdev = np.zeros((NV, 1024 + XPAD, 128), dtype=bf16)
        xdev[:, :1024] = x7.transpose(0, 1, 4, 2, 3, 5, 6).reshape(
            NV, 1024, 128
        ).astype(bf16)
        in_maps.append({"x": xdev, "wgblk": wgblk, "sel2": sel2})
    return in_maps


def kernel(edges, nodes, g_node, g_edge, W, b):
    edges = np.ascontiguousarray(edges, dtype=np.float32)
    nodes = np.ascontiguousarray(nodes, dtype=np.float32)
    g_node = np.asarray(g_node, dtype=np.float32)
    g_edge = np.asarray(g_edge, dtype=np.float32)
    W = np.asarray(W, dtype=np.float32)
    b = np.asarray(b, dtype=np.float32)

    # tiny node path on host (B*N*dn = 131K elems)
    ms = np.mean(np.square(nodes), axis=-1, keepdims=True)
    nodes_n = nodes / np.sqrt(ms + EPS) * g_node  # [B, N, 128]
    Wr, Wc, We = W[:DN], W[DN : 2 * DN], W[2 * DN :]
    row_proj = (nodes_n @ Wr).astype(np.float32)  # [B, N, 64]
    col_proj = (nodes_n @ Wc).astype(np.float32)  # [B, N, 64]

    in_maps = _make_in_maps(edges, g_edge, We)

    from concourse.bass_utils import run_bass_kernel_spmd

    nc = _get_nc()
    res = run_bass_kernel_spmd(nc, in_maps, list(range(NCORES)))

    out = np.empty((B, N, N, DE), dtype=np.float32)
    for c in range(NCORES):
        oc = res.results[c]["out"].astype(np.float32).reshape(B, IPC, N, DE)
        out[:, c * IPC : (c + 1) * IPC] = oc
    # node projections + bias added in f32 on the host
    out += row_proj[:, :, None, :] + b
    out += col_proj[:, None, :, :]
    return out


if __name__ == "__main__":
    rng = np.random.default_rng(0)
    edges = rng.standard_normal((B, N, N, DE), dtype=np.float32)
    nodes = rng.standard_normal((B, N, DN), dtype=np.float32)
    g_node = np.ones(DN, np.float32)
    g_edge = np.ones(DE, np.float32)
    W = rng.standard_normal((2 * DN + DE, DE), dtype=np.float32) / 18.0
    b = (rng.standard_normal(DE) * 0.01).astype(np.float32)
    o = kernel(edges, nodes, g_node, g_edge, W, b)
    print(o.shape, o.dtype)


# revision 5
# speedup vs baseline: 1.8435x; 1.0239x over previous
"""Bass/Trainium2 kernel for nn_NodesToEdges (gnn_message_passing).

out[b,i,j,:] = rms(edges[b,i,j,:])*g_e @ We + rms(nodes[b,i,:])*g_n @ Wr
             + rms(nodes[b,j,:])*g_n @ Wc + bias

Strategy: shard over i (rows) across 8 cores. The device computes ONLY
the heavy edge term y = rms(edges)*g_e @ We in bf16; the tiny node
projections + bias are added on the host in f32 (broadcast adds).

Device schedule, per SLOT of 4 blocks, working set [128 part, 1024]:

  xT loaded TRANSPOSED straight from DRAM via xbar transpose-DMAs
  (host pre-arranges x as [slot, (q a p), (r2 e)]; split SP 960 rows /
  ACT 64 rows); DVE squares xT (bf16 2x); PE reduces sumsq via 8
  tiny-moving matmuls (stat=sqT chunk, mov=2-col selector) ->
  psS[128,16]; ACT sqrt(mean+eps); DVE recip -> inv[128,16]; PE
  y-matmuls (stat=raw xT chunk, mov=blockdiag Wg) -> psE row-major
  f32; drain psE -> ysb bf16 split ACT/DVE by columns; Pool scales
  ysb in place by inv (per-edge rsqrt, SBUF-only so GPSIMD is legal);
  out-DMA of ysb rotates across Pool/SP/ACT queues (2/2/1 of 5).

Stage lags (iteration = slot + L): inT@0 square@1 {sumsq,y}@2
{sqrt,drain}@3 recip@4 scale@5 out@6.
"""

import numpy as np

B, N, DE, DN = 2, 512, 64, 128
NCORES = 8
IPC = N // NCORES          # 64 i-rows per core
NBLK = B * IPC             # 128 blocks of 512 rows per core
NV = NBLK // 4             # 32 slots of 4 blocks
EPS = float(np.finfo(np.float32).eps)

# tunables
DR_A = 192                 # drain: ACT raw [0:DR_A], DVE fused-scale rest
GR_A = DR_A // DE          # groups in the ACT share
XPAD = 64                  # xbar tail-tile guard rows (DMA sem can
                           # fire before the last ~4 tiles land)
DXT = 6                    # xT buffers
DSQ = 3
DRMS = 3
DINV = 4
DPE = 3                    # psE buffers (2 banks each)
DY = 10                    # ysb2 buffers (= out-queue rotation period)
DYR = 3                    # raw ysb buffers (ACT share only)
LOUT = 8                   # out(v) at iteration v+LOUT
# inT halves: SP rows [0:ISP_END], ACT rows [IAC_BEG:1088]; the overlap
# [IAC_BEG:ISP_END] is written identically by both DMAs, guarding SP's
# at-risk tail tiles; ACT's at-risk tail is the XPAD scratch.
ISP_END = 576
IAC_BEG = 512
# out queue by v%10: 8 sp, 1 act, 1 pool
OUTQ = ["sp", "act", "act", "act", "pool",
        "act", "sp", "act", "act", "act"]


def _build_nc(nv=NV):
    from contextlib import ExitStack

    import concourse.bass as bass
    import concourse.mybir as mybir

    f32 = mybir.dt.float32
    bf16 = mybir.dt.bfloat16
    SQRT = mybir.ActivationFunctionType.Sqrt

    nc = bass.Bass()
    nblk = 4 * nv
    x_d = nc.declare_dram_parameter("x", [nv, 1024 + XPAD, 128], bf16, isOutput=False)
    wgblk_d = nc.declare_dram_parameter("wgblk", [128, 128], bf16, isOutput=False)
    sel2_d = nc.declare_dram_parameter("sel2", [128, 2], bf16, isOutput=False)
    out_d = nc.declare_dram_parameter("out", [nblk, N, DE], bf16, isOutput=True)

    st = ExitStack()
    with st:
        sb = lambda shape, dt, name: st.enter_context(
            nc.sbuf_tensor(name, shape, dt)
        )
        wgblk = sb([128, 128], bf16, "wgblk_sb")
        sel2 = sb([128, 2], bf16, "sel2_sb")
        epsb = sb([128, 1], f32, "epsb")
        dum = sb([128, 1], f32, "dum")
        xT = [sb([128, 1024 + XPAD], bf16, f"xT{i}") for i in range(DXT)]
        sqT = [sb([128, 1024], bf16, f"sqT{i}") for i in range(DSQ)]
        rms = [sb([128, 16], f32, f"rms{i}") for i in range(DRMS)]
        inv = [sb([128, 16], f32, f"inv{i}") for i in range(DINV)]
        ysb = [sb([128, DR_A], bf16, f"ysb{i}") for i in range(DYR)]
        ysb2 = [sb([128, 1024], bf16, f"ysb2_{i}") for i in range(DY)]
        psS = [
            st.enter_context(nc.psum_tensor(f"psS{i}", [128, 512], f32))
            for i in range(2)
        ]
        psE = [
            st.enter_context(nc.psum_tensor(f"psE{i}", [128, 1024], f32))
            for i in range(DPE)
        ]

        sem = lambda name: st.enter_context(nc.semaphore(name))
        s_c = sem("s_c")
        s_cp = sem("s_cp")
        s_eps = sem("s_eps")
        s_in = [sem(f"s_in{i}") for i in range(DXT)]
        s_in2 = [sem(f"s_in2_{i}") for i in range(DXT)]
        s_sq = sem("s_sq")
        s_rms = sem("s_rms")
        s_inv = sem("s_inv")
        s_y = sem("s_y")
        s_dra = sem("s_dra")
        s_drd = sem("s_drd")
        s_sc = sem("s_sc")
        s_o = [sem(f"s_o{i}") for i in range(DY)]

        def out_ap(v):
            return (
                out_d[4 * v : 4 * v + 4]
                .rearrange("(s a) (p r) e -> s a p r e", s=2, p=64)
                .rearrange("s a p r e -> a p s r e")
                .rearrange("a p s r e -> (a p) s (r e)")
            )

        def emit_out(eng, w):
            eng.wait_ge(s_sc, w + 1)
            eng.wait_ge(s_drd, w + 1)
            eng.dma_start(
                out=out_ap(w),
                in_=ysb2[w % DY][:].rearrange("P (s f) -> P s f", s=2),
            ).then_inc(s_o[w % DY], 16)

        def wait_ysb2_free(eng, v):
            # ysb2[v % DY] was last read by out(v - DY)
            if v >= DY:
                w = v - DY
                eng.wait_ge(s_o[w % DY], 16 * (w // DY + 1))

        with nc.Block() as block:

            @block.sync
            def _(sync):
                for t in range(nv + LOUT):
                    # inT(t): whole slot (+ guard pad) in one xbar DMA
                    if t < nv:
                        if t >= DXT:
                            sync.wait_ge(s_y, t - DXT + 1)
                            sync.wait_ge(s_sq, t - DXT + 1)
                        sync.dma_start_transpose(
                            xT[t % DXT][:], x_d[t]
                        ).then_inc(s_in[t % DXT], 16)
                    if LOUT <= t < nv + LOUT and OUTQ[(t - LOUT) % DY] == "sp":
                        emit_out(sync, t - LOUT)

            @block.scalar
            def _(scalar):
                for cdst, csrc in ((wgblk, wgblk_d[:]), (sel2, sel2_d[:])):
                    scalar.dma_start(out=cdst[:], in_=csrc).then_inc(s_c, 16)
                # dummy Sqrt primes the ACT function table off the hot path
                scalar.wait_ge(s_c, 32)
                nc.scalar.activation(
                    dum[:], wgblk[:, :1], SQRT, bias=0.0, scale=0.0
                )

                for t in range(nv + LOUT):
                    # sqrt(t-5)
                    if 5 <= t < nv + 5:
                        w = t - 5
                        if w == 0:
                            scalar.wait_ge(s_eps, 1)
                        scalar.wait_ge(s_y, w + 1)
                        if w >= DRMS:
                            scalar.wait_ge(s_inv, w - DRMS + 1)
                        nc.scalar.activation(
                            rms[w % DRMS][:], psS[w % 2][:, :16], SQRT,
                            bias=epsb[:], scale=1.0 / DE,
                        ).then_inc(s_rms, 1)
                    # drain_a(t-6): cols [0:DR_A], raw psE -> ysb bf16
                    if 6 <= t < nv + 6:
                        w = t - 6
                        scalar.wait_ge(s_y, w + 1)
                        if w >= DYR:
                            scalar.wait_ge(s_sc, w - DYR + 1)
                        nc.scalar.copy(
                            ysb[w % DYR][:], psE[w % DPE][:, :DR_A]
                        ).then_inc(s_dra, 1)
                    if LOUT <= t < nv + LOUT and OUTQ[(t - LOUT) % DY] == "act":
                        emit_out(scalar, t - LOUT)

            @block.vector
            def _(vector):
                nc.vector.memset(epsb[:], EPS).then_inc(s_eps, 1)
                for t in range(nv + LOUT):
                    # recip(t-6)
                    if 6 <= t < nv + 6:
                        w = t - 6
                        vector.wait_ge(s_rms, w + 1)
                        if w >= DINV:
                            vector.wait_ge(s_sc, w - DINV + 1)
                            vector.wait_ge(s_drd, w - DINV + 1)
                        nc.vector.reciprocal(
                            inv[w % DINV][:], rms[w % DRMS][:]
                        ).then_inc(s_inv, 1)
                    # drain_d(t-7): fused drain+scale, cols [DR_A:1024]
                    if 7 <= t < nv + 7:
                        w = t - 7
                        vector.wait_ge(s_y, w + 1)
                        vector.wait_ge(s_inv, w + 1)
                        wait_ysb2_free(vector, w)
                        nc.vector.tensor_mul(
                            ysb2[w % DY][:, DR_A:]
                            .rearrange("p (g e) -> p g e", e=DE),
                            psE[w % DPE][:, DR_A:]
                            .rearrange("p (g e) -> p g e", e=DE),
                            inv[w % DINV][:, GR_A:]
                            .unsqueeze(-1)
                            .broadcast_to([128, 16 - GR_A, DE]),
                        ).then_inc(s_drd, 1)

            @block.gpsimd
            def _(pool):
                for t in range(nv + LOUT):
                    # square(t-3) on Pool (DVE self-operand mul is unsafe
                    # on HW; Pool's is baseline-proven)
                    if 3 <= t < nv + 3:
                        w = t - 3
                        pool.wait_ge(s_in[w % DXT], 16 * (w // DXT + 1))
                        if w >= DSQ:
                            pool.wait_ge(s_y, w - DSQ + 1)
                        nc.gpsimd.tensor_mul(
                            sqT[w % DSQ][:],
                            xT[w % DXT][:, :1024],
                            xT[w % DXT][:, :1024],
                        ).then_inc(s_sq, 1)
                    # scale_p(t-7): ysb2[:, :DR_A] = ysb * inv (not in-place)
                    if 7 <= t < nv + 7:
                        w = t - 7
                        pool.wait_ge(s_inv, w + 1)
                        pool.wait_ge(s_dra, w + 1)
                        wait_ysb2_free(pool, w)
                        nc.gpsimd.tensor_mul(
                            ysb2[w % DY][:, :DR_A]
                            .rearrange("p (g e) -> p g e", e=DE),
                            ysb[w % DYR][:]
                            .rearrange("p (g e) -> p g e", e=DE),
                            inv[w % DINV][:, :GR_A]
                            .unsqueeze(-1)
                            .broadcast_to([128, GR_A, DE]),
                        ).then_inc(s_sc, 1)
                    if LOUT <= t < nv + LOUT and OUTQ[(t - LOUT) % DY] == "pool":
                        emit_out(pool, t - LOUT)

            @block.tensor
            def _(tensor):
                tensor.wait_ge(s_c, 32)
                # warm-up matmul: starts the PE p-state ramp clock early so
                # the first real matmuls run at full frequency; its output in
                # psS[1] is zeroed by sumsq(1)'s group start before any read
                nc.tensor.matmul(
                    psS[1][:, :128], wgblk[:], wgblk[:],
                    start=True, stop=True, skip_group_check=True,
                )
                for t in range(nv + 4):
                    if 4 <= t < nv + 4:
                        w = t - 4
                        # sumsq: 8 tiny matmuls -> psS[:, 2q:2q+2]
                        tensor.wait_ge(s_sq, w + 1)
                        if w >= 2:
                            tensor.wait_ge(s_rms, w - 1)
                        for q in range(8):
                            nc.tensor.matmul(
                                psS[w % 2][:, 2 * q : 2 * q + 2],
                                sqT[w % DSQ][:, 128 * q : 128 * q + 128],
                                sel2[:],
                                start=(q == 0), stop=(q == 7),
                                skip_group_check=True,
                            )
                        # y: 8 matmuls raw xT vs blockdiag Wg
                        if w >= DPE:
                            tensor.wait_ge(s_dra, w - DPE + 1)
                            tensor.wait_ge(s_drd, w - DPE + 1)
                        # (xT also read by Pool square; inT waits s_sq too)
                        for q in range(8):
                            mm = nc.tensor.matmul(
                                psE[w % DPE][:, 128 * q : 128 * q + 128],
                                xT[w % DXT][:, 128 * q : 128 * q + 128],
                                wgblk[:],
                                start=(q % 4 == 0), stop=(q % 4 == 3),
                                skip_group_check=True,
                            )
                            if q == 7:
                                mm.then_inc(s_y, 1)

    return nc


_NC_CACHE = {}


def _get_nc():
    if "nc" not in _NC_CACHE:
        _NC_CACHE["nc"] = _build_nc()
    return _NC_CACHE["nc"]


def _make_in_maps(edges, g_edge, We):
    import ml_dtypes

    bf16 = ml_dtypes.bfloat16
    Wg = (np.asarray(g_edge, dtype=np.float32)[:, None] * We).astype(np.float32)

    wgblk = np.zeros((128, 128), dtype=bf16)
    wgblk[:64, :64] = Wg.astype(bf16)
    wgblk[64:, 64:] = Wg.astype(bf16)
    sel2 = np.zeros((128, 2), dtype=bf16)
    sel2[:64, 0] = 1
    sel2[64:, 1] = 1

    in_maps = []
    for c in range(NCORES):
        # x device layout: [nv, 1024=(q:(s,rp), a, p), 128=(r2, e)]
        xs = edges[:, c * IPC : (c + 1) * IPC]  # [B, 64, 512, 64]
        xs = np.ascontiguousarray(xs).reshape(NBLK, N, DE)
        x7 = xs.reshape(NV, 2, 2, 64, 4, 2, 64)  # v s a p rp r2 e
        xdev = np.zeros((NV, 1024 + XPAD, 128), dtype=bf16)
        xdev[:, :1024] = x7.transpose(0, 1, 4, 2, 3, 5, 6).reshape(
            NV, 1024, 128
        ).astype(bf16)
        in_maps.append({"x": xdev, "wgblk": wgblk, "sel2": sel2})
    return in_maps


def kernel(edges, nodes, g_node, g_edge, W, b):
    edges = np.ascontiguousarray(edges, dtype=np.float32)
    nodes = np.ascontiguousarray(nodes, dtype=np.float32)
    g_node = np.asarray(g_node, dtype=np.float32)
    g_edge = np.asarray(g_edge, dtype=np.float32)
    W = np.asarray(W, dtype=np.float32)
    b = np.asarray(b, dtype=np.float32)

    # tiny node path on host (B*N*dn = 131K elems)
    ms = np.mean(np.square(nodes), axis=-1, keepdims=True)
    nodes_n = nodes / np.sqrt(ms + EPS) * g_node  # [B, N, 128]
    Wr, Wc, We = W[:DN], W[DN : 2 * DN], W[2 * DN :]
    row_proj = (nodes_n @ Wr).astype(np.float32)  # [B, N, 64]
    col_proj = (nodes_n @ Wc).astype(np.float32)  # [B, N, 64]

    in_maps = _make_in_maps(edges, g_edge, We)

    from concourse.bass_utils import run_bass_kernel_spmd

    nc = _get_nc()
    res = run_bass_kernel_spmd(nc, in_maps, list(range(NCORES)))

    out = np.empty((B, N, N, DE), dtype=np.float32)
    for c in range(NCORES):
        oc = res.results[c]["out"].astype(np.float32).reshape(B, IPC, N, DE)
        out[:, c * IPC : (c + 1) * IPC] = oc
    # node projections + bias added in f32 on the host
    out += row_proj[:, :, None, :] + b
    out += col_proj[:, None, :, :]
    return out


if __name__ == "__main__":
    rng = np.random.default_rng(0)
    edges = rng.standard_normal((B, N, N, DE), dtype=np.float32)
    nodes = rng.standard_normal((B, N, DN), dtype=np.float32)
    g_node = np.ones(DN, np.float32)
    g_edge = np.ones(DE, np.float32)
    W = rng.standard_normal((2 * DN + DE, DE), dtype=np.float32) / 18.0
    b = (rng.standard_normal(DE) * 0.01).astype(np.float32)
    o = kernel(edges, nodes, g_node, g_edge, W, b)
    print(o.shape, o.dtype)


# revision 6
# speedup vs baseline: 1.8761x; 1.0177x over previous
"""Bass/Trainium2 kernel for nn_NodesToEdges (gnn_message_passing).

out[b,i,j,:] = rms(edges[b,i,j,:])*g_e @ We + rms(nodes[b,i,:])*g_n @ Wr
             + rms(nodes[b,j,:])*g_n @ Wc + bias

Strategy: shard over i (rows) across 8 cores. The device computes ONLY
the heavy edge term y = rms(edges)*g_e @ We in bf16; the tiny node
projections + bias are added on the host in f32 (broadcast adds).

Device schedule, per SLOT of 4 blocks, working set [128 part, 1024]:

  xT loaded TRANSPOSED straight from DRAM via xbar transpose-DMAs
  (host pre-arranges x as [slot, (q a p), (r2 e)]; split SP 960 rows /
  ACT 64 rows); DVE squares xT (bf16 2x); PE reduces sumsq via 8
  tiny-moving matmuls (stat=sqT chunk, mov=2-col selector) ->
  psS[128,16]; ACT sqrt(mean+eps); DVE recip -> inv[128,16]; PE
  y-matmuls (stat=raw xT chunk, mov=blockdiag Wg) -> psE row-major
  f32; drain psE -> ysb bf16 split ACT/DVE by columns; Pool scales
  ysb in place by inv (per-edge rsqrt, SBUF-only so GPSIMD is legal);
  out-DMA of ysb rotates across Pool/SP/ACT queues (2/2/1 of 5).

Stage lags (iteration = slot + L): inT@0 square@1 {sumsq,y}@2
{sqrt,drain}@3 recip@4 scale@5 out@6.
"""

import numpy as np

B, N, DE, DN = 2, 512, 64, 128
NCORES = 8
IPC = N // NCORES          # 64 i-rows per core
NBLK = B * IPC             # 128 blocks of 512 rows per core
NV = NBLK // 4             # 32 slots of 4 blocks
EPS = float(np.finfo(np.float32).eps)

# tunables
DR_A = 192                 # drain: ACT raw [0:DR_A], DVE fused-scale rest
GR_A = DR_A // DE          # groups in the ACT share
XPAD = 64                  # xbar tail-tile guard rows (DMA sem can
                           # fire before the last ~4 tiles land)
DXT = 6                    # xT buffers
DSQ = 3
DRMS = 3
DINV = 4
DPE = 3                    # psE buffers (2 banks each)
DY = 10                    # ysb2 buffers (= out-queue rotation period)
DYR = 3                    # raw ysb buffers (ACT share only)
LOUT = 8                   # out(v) at iteration v+LOUT
# inT halves: SP rows [0:ISP_END], ACT rows [IAC_BEG:1088]; the overlap
# [IAC_BEG:ISP_END] is written identically by both DMAs, guarding SP's
# at-risk tail tiles; ACT's at-risk tail is the XPAD scratch.
ISP_END = 576
IAC_BEG = 512
# out queue by v%10: 8 sp, 1 act, 1 pool
OUTQ = ["sp", "act", "act", "sp", "pool",
        "act", "sp", "act", "pool", "act"]


def _build_nc(nv=NV):
    from contextlib import ExitStack

    import concourse.bass as bass
    import concourse.mybir as mybir

    f32 = mybir.dt.float32
    bf16 = mybir.dt.bfloat16
    SQRT = mybir.ActivationFunctionType.Sqrt

    nc = bass.Bass()
    nblk = 4 * nv
    x_d = nc.declare_dram_parameter("x", [nv, 1024 + XPAD, 128], bf16, isOutput=False)
    wgblk_d = nc.declare_dram_parameter("wgblk", [128, 128], bf16, isOutput=False)
    sel2_d = nc.declare_dram_parameter("sel2", [128, 2], bf16, isOutput=False)
    out_d = nc.declare_dram_parameter("out", [nblk, N, DE], bf16, isOutput=True)

    st = ExitStack()
    with st:
        sb = lambda shape, dt, name: st.enter_context(
            nc.sbuf_tensor(name, shape, dt)
        )
        wgblk = sb([128, 128], bf16, "wgblk_sb")
        sel2 = sb([128, 2], bf16, "sel2_sb")
        epsb = sb([128, 1], f32, "epsb")
        dum = sb([128, 1], f32, "dum")
        xT = [sb([128, 1024 + XPAD], bf16, f"xT{i}") for i in range(DXT)]
        sqT = [sb([128, 1024], bf16, f"sqT{i}") for i in range(DSQ)]
        rms = [sb([128, 16], f32, f"rms{i}") for i in range(DRMS)]
        inv = [sb([128, 16], f32, f"inv{i}") for i in range(DINV)]
        ysb = [sb([128, DR_A], bf16, f"ysb{i}") for i in range(DYR)]
        ysb2 = [sb([128, 1024], bf16, f"ysb2_{i}") for i in range(DY)]
        psS = [
            st.enter_context(nc.psum_tensor(f"psS{i}", [128, 512], f32))
            for i in range(2)
        ]
        psE = [
            st.enter_context(nc.psum_tensor(f"psE{i}", [128, 1024], f32))
            for i in range(DPE)
        ]

        sem = lambda name: st.enter_context(nc.semaphore(name))
        s_c = sem("s_c")
        s_cp = sem("s_cp")
        s_eps = sem("s_eps")
        s_in = [sem(f"s_in{i}") for i in range(DXT)]
        s_in2 = [sem(f"s_in2_{i}") for i in range(DXT)]
        s_sq = sem("s_sq")
        s_rms = sem("s_rms")
        s_inv = sem("s_inv")
        s_y = sem("s_y")
        s_dra = sem("s_dra")
        s_drd = sem("s_drd")
        s_sc = sem("s_sc")
        s_o = [sem(f"s_o{i}") for i in range(DY)]

        def out_ap(v):
            return (
                out_d[4 * v : 4 * v + 4]
                .rearrange("(s a) (p r) e -> s a p r e", s=2, p=64)
                .rearrange("s a p r e -> a p s r e")
                .rearrange("a p s r e -> (a p) s (r e)")
            )

        def emit_out(eng, w):
            eng.wait_ge(s_sc, w + 1)
            eng.wait_ge(s_drd, w + 1)
            eng.dma_start(
                out=out_ap(w),
                in_=ysb2[w % DY][:].rearrange("P (s f) -> P s f", s=2),
            ).then_inc(s_o[w % DY], 16)

        def wait_ysb2_free(eng, v):
            # ysb2[v % DY] was last read by out(v - DY)
            if v >= DY:
                w = v - DY
                eng.wait_ge(s_o[w % DY], 16 * (w // DY + 1))

        with nc.Block() as block:

            @block.sync
            def _(sync):
                for t in range(nv + LOUT):
                    # inT(t): whole slot (+ guard pad) in one xbar DMA
                    if t < nv:
                        if t >= DXT:
                            sync.wait_ge(s_y, t - DXT + 1)
                            sync.wait_ge(s_sq, t - DXT + 1)
                        sync.dma_start_transpose(
                            xT[t % DXT][:], x_d[t]
                        ).then_inc(s_in[t % DXT], 16)
                    if LOUT <= t < nv + LOUT and OUTQ[(t - LOUT) % DY] == "sp":
                        emit_out(sync, t - LOUT)

            @block.scalar
            def _(scalar):
                for cdst, csrc in ((wgblk, wgblk_d[:]), (sel2, sel2_d[:])):
                    scalar.dma_start(out=cdst[:], in_=csrc).then_inc(s_c, 16)
                # dummy Sqrt primes the ACT function table off the hot path
                scalar.wait_ge(s_c, 32)
                nc.scalar.activation(
                    dum[:], wgblk[:, :1], SQRT, bias=0.0, scale=0.0
                )

                for t in range(nv + LOUT):
                    # sqrt(t-5)
                    if 5 <= t < nv + 5:
                        w = t - 5
                        if w == 0:
                            scalar.wait_ge(s_eps, 1)
                        scalar.wait_ge(s_y, w + 1)
                        if w >= DRMS:
                            scalar.wait_ge(s_inv, w - DRMS + 1)
                        nc.scalar.activation(
                            rms[w % DRMS][:], psS[w % 2][:, :16], SQRT,
                            bias=epsb[:], scale=1.0 / DE,
                        ).then_inc(s_rms, 1)
                    # drain_a(t-6): cols [0:DR_A], raw psE -> ysb bf16
                    if 6 <= t < nv + 6:
                        w = t - 6
                        scalar.wait_ge(s_y, w + 1)
                        if w >= DYR:
                            scalar.wait_ge(s_sc, w - DYR + 1)
                        nc.scalar.copy(
                            ysb[w % DYR][:], psE[w % DPE][:, :DR_A]
                        ).then_inc(s_dra, 1)
                    if LOUT <= t < nv + LOUT and OUTQ[(t - LOUT) % DY] == "act":
                        emit_out(scalar, t - LOUT)

            @block.vector
            def _(vector):
                nc.vector.memset(epsb[:], EPS).then_inc(s_eps, 1)
                for t in range(nv + LOUT):
                    # recip(t-6)
                    if 6 <= t < nv + 6:
                        w = t - 6
                        vector.wait_ge(s_rms, w + 1)
                        if w >= DINV:
                            vector.wait_ge(s_sc, w - DINV + 1)
                            vector.wait_ge(s_drd, w - DINV + 1)
                        nc.vector.reciprocal(
                            inv[w % DINV][:], rms[w % DRMS][:]
                        ).then_inc(s_inv, 1)
                    # drain_d(t-7): fused drain+scale, cols [DR_A:1024]
                    if 7 <= t < nv + 7:
                        w = t - 7
                        vector.wait_ge(s_y, w + 1)
                        vector.wait_ge(s_inv, w + 1)
                        wait_ysb2_free(vector, w)
                        nc.vector.tensor_mul(
                            ysb2[w % DY][:, DR_A:]
                            .rearrange("p (g e) -> p g e", e=DE),
                            psE[w % DPE][:, DR_A:]
                            .rearrange("p (g e) -> p g e", e=DE),
                            inv[w % DINV][:, GR_A:]
                            .unsqueeze(-1)
                            .broadcast_to([128, 16 - GR_A, DE]),
                        ).then_inc(s_drd, 1)

            @block.gpsimd
            def _(pool):
                for t in range(nv + LOUT):
                    # square(t-3) on Pool (DVE self-operand mul is unsafe
                    # on HW; Pool's is baseline-proven)
                    if 3 <= t < nv + 3:
                        w = t - 3
                        pool.wait_ge(s_in[w % DXT], 16 * (w // DXT + 1))
                        if w >= DSQ:
                            pool.wait_ge(s_y, w - DSQ + 1)
                        nc.gpsimd.tensor_mul(
                            sqT[w % DSQ][:],
                            xT[w % DXT][:, :1024],
                            xT[w % DXT][:, :1024],
                        ).then_inc(s_sq, 1)
                    # scale_p(t-7): ysb2[:, :DR_A] = ysb * inv (not in-place)
                    if 7 <= t < nv + 7:
                        w = t - 7
                        pool.wait_ge(s_inv, w + 1)
                        pool.wait_ge(s_dra, w + 1)
                        wait_ysb2_free(pool, w)
                        nc.gpsimd.tensor_mul(
                            ysb2[w % DY][:, :DR_A]
                            .rearrange("p (g e) -> p g e", e=DE),
                            ysb[w % DYR][:]
                            .rearrange("p (g e) -> p g e", e=DE),
                            inv[w % DINV][:, :GR_A]
                            .unsqueeze(-1)
                            .broadcast_to([128, GR_A, DE]),
                        ).then_inc(s_sc, 1)
                    if LOUT <= t < nv + LOUT and OUTQ[(t - LOUT) % DY] == "pool":
                        emit_out(pool, t - LOUT)

            @block.tensor
            def _(tensor):
                tensor.wait_ge(s_c, 32)
                # warm-up matmul: starts the PE p-state ramp clock early so
                # the first real matmuls run at full frequency; its output in
                # psS[1] is zeroed by sumsq(1)'s group start before any read
                nc.tensor.matmul(
                    psS[1][:, :128], wgblk[:], wgblk[:],
                    start=True, stop=True, skip_group_check=True,
                )
                for t in range(nv + 4):
                    if 4 <= t < nv + 4:
                        w = t - 4
                        # sumsq: 8 tiny matmuls -> psS[:, 2q:2q+2]
                        tensor.wait_ge(s_sq, w + 1)
                        if w >= 2:
                            tensor.wait_ge(s_rms, w - 1)
                        for q in range(8):
                            nc.tensor.matmul(
                                psS[w % 2][:, 2 * q : 2 * q + 2],
                                sqT[w % DSQ][:, 128 * q : 128 * q + 128],
                                sel2[:],
                                start=(q == 0), stop=(q == 7),
                                skip_group_check=True,
                            )
                        # y: 8 matmuls raw xT vs blockdiag Wg
                        if w >= DPE:
                            tensor.wait_ge(s_dra, w - DPE + 1)
                            tensor.wait_ge(s_drd, w - DPE + 1)
                        # (xT also read by Pool square; inT waits s_sq too)
                        for q in range(8):
                            mm = nc.tensor.matmul(
                                psE[w % DPE][:, 128 * q : 128 * q + 128],
                                xT[w % DXT][:, 128 * q : 128 * q + 128],
                                wgblk[:],
                                start=(q % 4 == 0), stop=(q % 4 == 3),
                                skip_group_check=True,
                            )
                            if q == 7:
                                mm.then_inc(s_y, 1)

    return nc


_NC_CACHE = {}


def _get_nc():
    if "nc" not in _NC_CACHE:
        _NC_CACHE["nc"] = _build_nc()
    return _NC_CACHE["nc"]


def _make_in_maps(edges, g_edge, We):
    import ml_dtypes

    bf16 = ml_dtypes.bfloat16
    Wg = (np.asarray(g_edge, dtype=np.float32)[:, None] * We).astype(np.float32)

    wgblk = np.zeros((128, 128), dtype=bf16)
    wgblk[:64, :64] = Wg.astype(bf16)
    wgblk[64:, 64:] = Wg.astype(bf16)
    sel2 = np.zeros((128, 2), dtype=bf16)
    sel2[:64, 0] = 1
    sel2[64:, 1] = 1

    in_maps = []
    for c in range(NCORES):
        # x device layout: [nv, 1024=(q:(s,rp), a, p), 128=(r2, e)]
        xs = edges[:, c * IPC : (c + 1) * IPC]  # [B, 64, 512, 64]
        xs = np.ascontiguousarray(xs).reshape(NBLK, N, DE)
        x7 = xs.reshape(NV, 2, 2, 64, 4, 2, 64)  # v s a p rp r2 e
        xdev = np.zeros((NV, 1024 + XPAD, 128), dtype=bf16)
        xdev[:, :1024] = x7.transpose(0, 1, 4, 2, 3, 5, 6).reshape(
            NV, 1024, 128
        ).astype(bf16)
        in_maps.append({"x": xdev, "wgblk": wgblk, "sel2": sel2})
    return in_maps


def kernel(edges, nodes, g_node, g_edge, W, b):
    edges = np.ascontiguousarray(edges, dtype=np.float32)
    nodes = np.ascontiguousarray(nodes, dtype=np.float32)
    g_node = np.asarray(g_node, dtype=np.float32)
    g_edge = np.asarray(g_edge, dtype=np.float32)
    W = np.asarray(W, dtype=np.float32)
    b = np.asarray(b, dtype=np.float32)

    # tiny node path on host (B*N*dn = 131K elems)
    ms = np.mean(np.square(nodes), axis=-1, keepdims=True)
    nodes_n = nodes / np.sqrt(ms + EPS) * g_node  # [B, N, 128]
    Wr, Wc, We = W[:DN], W[DN : 2 * DN], W[2 * DN :]
    row_proj = (nodes_n @ Wr).astype(np.float32)  # [B, N, 64]
    col_proj = (nodes_n @ Wc).astype(np.float32)  # [B, N, 64]

    in_maps = _make_in_maps(edges, g_edge, We)

    from concourse.bass_utils import run_bass_kernel_spmd

    nc = _get_nc()
    res = run_bass_kernel_spmd(nc, in_maps, list(range(NCORES)))

    out = np.empty((B, N, N, DE), dtype=np.float32)
    for c in range(NCORES):
        oc = res.results[c]["out"].astype(np.float32).reshape(B, IPC, N, DE)
        out[:, c * IPC : (c + 1) * IPC] = oc
    # node projections + bias added in f32 on the host
    out += row_proj[:, :, None, :] + b
    out += col_proj[:, None, :, :]
    return out


if __name__ == "__main__":
    rng = np.random.default_rng(0)
    edges = rng.standard_normal((B, N, N, DE), dtype=np.float32)
    nodes = rng.standard_normal((B, N, DN), dtype=np.float32)
    g_node = np.ones(DN, np.float32)
    g_edge = np.ones(DE, np.float32)
    W = rng.standard_normal((2 * DN + DE, DE), dtype=np.float32) / 18.0
    b = (rng.standard_normal(DE) * 0.01).astype(np.float32)
    o = kernel(edges, nodes, g_node, g_edge, W, b)
    print(o.shape, o.dtype)


# revision 7
# speedup vs baseline: 1.8784x; 1.0013x over previous
"""Bass/Trainium2 kernel for nn_NodesToEdges (gnn_message_passing).

out[b,i,j,:] = rms(edges[b,i,j,:])*g_e @ We + rms(nodes[b,i,:])*g_n @ Wr
             + rms(nodes[b,j,:])*g_n @ Wc + bias

Strategy: shard over i (rows) across 8 cores. The device computes ONLY
the heavy edge term y = rms(edges)*g_e @ We in bf16; the tiny node
projections + bias are added on the host in f32 (broadcast adds).

Device schedule, per SLOT of 4 blocks, working set [128 part, 1024]:

  xT loaded TRANSPOSED straight from DRAM via xbar transpose-DMAs
  (host pre-arranges x as [slot, (q a p), (r2 e)]; split SP 960 rows /
  ACT 64 rows); DVE squares xT (bf16 2x); PE reduces sumsq via 8
  tiny-moving matmuls (stat=sqT chunk, mov=2-col selector) ->
  psS[128,16]; ACT sqrt(mean+eps); DVE recip -> inv[128,16]; PE
  y-matmuls (stat=raw xT chunk, mov=blockdiag Wg) -> psE row-major
  f32; drain psE -> ysb bf16 split ACT/DVE by columns; Pool scales
  ysb in place by inv (per-edge rsqrt, SBUF-only so GPSIMD is legal);
  out-DMA of ysb rotates across Pool/SP/ACT queues (2/2/1 of 5).

Stage lags (iteration = slot + L): inT@0 square@1 {sumsq,y}@2
{sqrt,drain}@3 recip@4 scale@5 out@6.
"""

import numpy as np

B, N, DE, DN = 2, 512, 64, 128
NCORES = 8
IPC = N // NCORES          # 64 i-rows per core
NBLK = B * IPC             # 128 blocks of 512 rows per core
NV = NBLK // 4             # 32 slots of 4 blocks
EPS = float(np.finfo(np.float32).eps)

# tunables
DR_A = 192                 # drain: ACT raw [0:DR_A], DVE fused-scale rest
GR_A = DR_A // DE          # groups in the ACT share
XPAD = 64                  # xbar tail-tile guard rows (DMA sem can
                           # fire before the last ~4 tiles land)
DXT = 6                    # xT buffers
DSQ = 3
DRMS = 3
DINV = 4
DPE = 3                    # psE buffers (2 banks each)
DY = 10                    # ysb2 buffers (= out-queue rotation period)
DYR = 3                    # raw ysb buffers (ACT share only)
LOUT = 9                   # out(v) at iteration v+LOUT
# inT halves: SP rows [0:ISP_END], ACT rows [IAC_BEG:1088]; the overlap
# [IAC_BEG:ISP_END] is written identically by both DMAs, guarding SP's
# at-risk tail tiles; ACT's at-risk tail is the XPAD scratch.
ISP_END = 576
IAC_BEG = 512
# out queue by v%10: 8 sp, 1 act, 1 pool
OUTQ = ["sp", "act", "act", "sp", "pool",
        "act", "sp", "act", "pool", "act"]


def _build_nc(nv=NV):
    from contextlib import ExitStack

    import concourse.bass as bass
    import concourse.mybir as mybir

    f32 = mybir.dt.float32
    bf16 = mybir.dt.bfloat16
    SQRT = mybir.ActivationFunctionType.Sqrt

    nc = bass.Bass()
    nblk = 4 * nv
    x_d = nc.declare_dram_parameter("x", [nv, 1024 + XPAD, 128], bf16, isOutput=False)
    wgblk_d = nc.declare_dram_parameter("wgblk", [128, 128], bf16, isOutput=False)
    sel2_d = nc.declare_dram_parameter("sel2", [128, 2], bf16, isOutput=False)
    out_d = nc.declare_dram_parameter("out", [nblk, N, DE], bf16, isOutput=True)

    st = ExitStack()
    with st:
        sb = lambda shape, dt, name: st.enter_context(
            nc.sbuf_tensor(name, shape, dt)
        )
        wgblk = sb([128, 128], bf16, "wgblk_sb")
        sel2 = sb([128, 2], bf16, "sel2_sb")
        epsb = sb([128, 1], f32, "epsb")
        dum = sb([128, 1], f32, "dum")
        xT = [sb([128, 1024 + XPAD], bf16, f"xT{i}") for i in range(DXT)]
        sqT = [sb([128, 1024], bf16, f"sqT{i}") for i in range(DSQ)]
        rms = [sb([128, 16], f32, f"rms{i}") for i in range(DRMS)]
        inv = [sb([128, 16], f32, f"inv{i}") for i in range(DINV)]
        ysb = [sb([128, DR_A], bf16, f"ysb{i}") for i in range(DYR)]
        ysb2 = [sb([128, 1024], bf16, f"ysb2_{i}") for i in range(DY)]
        psS = [
            st.enter_context(nc.psum_tensor(f"psS{i}", [128, 512], f32))
            for i in range(2)
        ]
        psE = [
            st.enter_context(nc.psum_tensor(f"psE{i}", [128, 1024], f32))
            for i in range(DPE)
        ]

        sem = lambda name: st.enter_context(nc.semaphore(name))
        s_c = sem("s_c")
        s_cp = sem("s_cp")
        s_eps = sem("s_eps")
        s_in = [sem(f"s_in{i}") for i in range(DXT)]
        s_in2 = [sem(f"s_in2_{i}") for i in range(DXT)]
        s_sq = sem("s_sq")
        s_rms = sem("s_rms")
        s_inv = sem("s_inv")
        s_y = sem("s_y")
        s_dra = sem("s_dra")
        s_drd = sem("s_drd")
        s_sc = sem("s_sc")
        s_o = [sem(f"s_o{i}") for i in range(DY)]

        def out_ap(v):
            return (
                out_d[4 * v : 4 * v + 4]
                .rearrange("(s a) (p r) e -> s a p r e", s=2, p=64)
                .rearrange("s a p r e -> a p s r e")
                .rearrange("a p s r e -> (a p) s (r e)")
            )

        def emit_out(eng, w):
            eng.wait_ge(s_sc, w + 1)
            eng.wait_ge(s_drd, w + 1)
            eng.dma_start(
                out=out_ap(w),
                in_=ysb2[w % DY][:].rearrange("P (s f) -> P s f", s=2),
            ).then_inc(s_o[w % DY], 16)

        def wait_ysb2_free(eng, v):
            # ysb2[v % DY] was last read by out(v - DY)
            if v >= DY:
                w = v - DY
                eng.wait_ge(s_o[w % DY], 16 * (w // DY + 1))

        with nc.Block() as block:

            @block.sync
            def _(sync):
                for t in range(nv + LOUT):
                    # inT(t): whole slot (+ guard pad) in one xbar DMA
                    if t < nv:
                        if t >= DXT:
                            sync.wait_ge(s_y, t - DXT + 1)
                            sync.wait_ge(s_sq, t - DXT + 1)
                        sync.dma_start_transpose(
                            xT[t % DXT][:], x_d[t]
                        ).then_inc(s_in[t % DXT], 16)
                    if LOUT <= t < nv + LOUT and OUTQ[(t - LOUT) % DY] == "sp":
                        emit_out(sync, t - LOUT)

            @block.scalar
            def _(scalar):
                for cdst, csrc in ((wgblk, wgblk_d[:]), (sel2, sel2_d[:])):
                    scalar.dma_start(out=cdst[:], in_=csrc).then_inc(s_c, 16)
                # dummy Sqrt primes the ACT function table off the hot path
                scalar.wait_ge(s_c, 32)
                nc.scalar.activation(
                    dum[:], wgblk[:, :1], SQRT, bias=0.0, scale=0.0
                )

                for t in range(nv + LOUT):
                    # sqrt(t-5)
                    if 5 <= t < nv + 5:
                        w = t - 5
                        if w == 0:
                            scalar.wait_ge(s_eps, 1)
                        scalar.wait_ge(s_y, w + 1)
                        if w >= DRMS:
                            scalar.wait_ge(s_inv, w - DRMS + 1)
                        nc.scalar.activation(
                            rms[w % DRMS][:], psS[w % 2][:, :16], SQRT,
                            bias=epsb[:], scale=1.0 / DE,
                        ).then_inc(s_rms, 1)
                    # drain_a(t-6): cols [0:DR_A], raw psE -> ysb bf16
                    if 6 <= t < nv + 6:
                        w = t - 6
                        scalar.wait_ge(s_y, w + 1)
                        if w >= DYR:
                            scalar.wait_ge(s_sc, w - DYR + 1)
                        nc.scalar.copy(
                            ysb[w % DYR][:], psE[w % DPE][:, :DR_A]
                        ).then_inc(s_dra, 1)
                    if LOUT <= t < nv + LOUT and OUTQ[(t - LOUT) % DY] == "act":
                        emit_out(scalar, t - LOUT)

            @block.vector
            def _(vector):
                nc.vector.memset(epsb[:], EPS).then_inc(s_eps, 1)
                for t in range(nv + LOUT):
                    # recip(t-6)
                    if 6 <= t < nv + 6:
                        w = t - 6
                        vector.wait_ge(s_rms, w + 1)
                        if w >= DINV:
                            vector.wait_ge(s_sc, w - DINV + 1)
                            vector.wait_ge(s_drd, w - DINV + 1)
                        nc.vector.reciprocal(
                            inv[w % DINV][:], rms[w % DRMS][:]
                        ).then_inc(s_inv, 1)
                    # drain_d(t-7): fused drain+scale, cols [DR_A:1024]
                    if 7 <= t < nv + 7:
                        w = t - 7
                        vector.wait_ge(s_y, w + 1)
                        vector.wait_ge(s_inv, w + 1)
                        wait_ysb2_free(vector, w)
                        nc.vector.tensor_mul(
                            ysb2[w % DY][:, DR_A:]
                            .rearrange("p (g e) -> p g e", e=DE),
                            psE[w % DPE][:, DR_A:]
                            .rearrange("p (g e) -> p g e", e=DE),
                            inv[w % DINV][:, GR_A:]
                            .unsqueeze(-1)
                            .broadcast_to([128, 16 - GR_A, DE]),
                        ).then_inc(s_drd, 1)

            @block.gpsimd
            def _(pool):
                for t in range(nv + LOUT):
                    # square(t-3) on Pool (DVE self-operand mul is unsafe
                    # on HW; Pool's is baseline-proven)
                    if 3 <= t < nv + 3:
                        w = t - 3
                        pool.wait_ge(s_in[w % DXT], 16 * (w // DXT + 1))
                        if w >= DSQ:
                            pool.wait_ge(s_y, w - DSQ + 1)
                        nc.gpsimd.tensor_mul(
                            sqT[w % DSQ][:],
                            xT[w % DXT][:, :1024],
                            xT[w % DXT][:, :1024],
                        ).then_inc(s_sq, 1)
                    # scale_p(t-7): ysb2[:, :DR_A] = ysb * inv (not in-place)
                    if 7 <= t < nv + 7:
                        w = t - 7
                        pool.wait_ge(s_inv, w + 1)
                        pool.wait_ge(s_dra, w + 1)
                        wait_ysb2_free(pool, w)
                        nc.gpsimd.tensor_mul(
                            ysb2[w % DY][:, :DR_A]
                            .rearrange("p (g e) -> p g e", e=DE),
                            ysb[w % DYR][:]
                            .rearrange("p (g e) -> p g e", e=DE),
                            inv[w % DINV][:, :GR_A]
                            .unsqueeze(-1)
                            .broadcast_to([128, GR_A, DE]),
                        ).then_inc(s_sc, 1)
                    if LOUT <= t < nv + LOUT and OUTQ[(t - LOUT) % DY] == "pool":
                        emit_out(pool, t - LOUT)

            @block.tensor
            def _(tensor):
                tensor.wait_ge(s_c, 32)
                # warm-up matmul: starts the PE p-state ramp clock early so
                # the first real matmuls run at full frequency; its output in
                # psS[1] is zeroed by sumsq(1)'s group start before any read
                nc.tensor.matmul(
                    psS[1][:, :128], wgblk[:], wgblk[:],
                    start=True, stop=True, skip_group_check=True,
                )
                for t in range(nv + 4):
                    if 4 <= t < nv + 4:
                        w = t - 4
                        # sumsq: 8 tiny matmuls -> psS[:, 2q:2q+2]
                        tensor.wait_ge(s_sq, w + 1)
                        if w >= 2:
                            tensor.wait_ge(s_rms, w - 1)
                        for q in range(8):
                            nc.tensor.matmul(
                                psS[w % 2][:, 2 * q : 2 * q + 2],
                                sqT[w % DSQ][:, 128 * q : 128 * q + 128],
                                sel2[:],
                                start=(q == 0), stop=(q == 7),
                                skip_group_check=True,
                            )
                        # y: 8 matmuls raw xT vs blockdiag Wg
                        if w >= DPE:
                            tensor.wait_ge(s_dra, w - DPE + 1)
                            tensor.wait_ge(s_drd, w - DPE + 1)
                        # (xT also read by Pool square; inT waits s_sq too)
                        for q in range(8):
                            mm = nc.tensor.matmul(
                                psE[w % DPE][:, 128 * q : 128 * q + 128],
                                xT[w % DXT][:, 128 * q : 128 * q + 128],
                                wgblk[:],
                                start=(q % 4 == 0), stop=(q % 4 == 3),
                                skip_group_check=True,
                            )
                            if q == 7:
                                mm.then_inc(s_y, 1)

    return nc


_NC_CACHE = {}


def _get_nc():
    if "nc" not in _NC_CACHE:
        _NC_CACHE["nc"] = _build_nc()
    return _NC_CACHE["nc"]


def _make_in_maps(edges, g_edge, We):
    import ml_dtypes

    bf16 = ml_dtypes.bfloat16
    Wg = (np.asarray(g_edge, dtype=np.float32)[:, None] * We).astype(np.float32)

    wgblk = np.zeros((128, 128), dtype=bf16)
    wgblk[:64, :64] = Wg.astype(bf16)
    wgblk[64:, 64:] = Wg.astype(bf16)
    sel2 = np.zeros((128, 2), dtype=bf16)
    sel2[:64, 0] = 1
    sel2[64:, 1] = 1

    in_maps = []
    for c in range(NCORES):
        # x device layout: [nv, 1024=(q:(s,rp), a, p), 128=(r2, e)]
        xs = edges[:, c * IPC : (c + 1) * IPC]  # [B, 64, 512, 64]
        xs = np.ascontiguousarray(xs).reshape(NBLK, N, DE)
        x7 = xs.reshape(NV, 2, 2, 64, 4, 2, 64)  # v s a p rp r2 e
        xdev = np.zeros((NV, 1024 + XPAD, 128), dtype=bf16)
        xdev[:, :1024] = x7.transpose(0, 1, 4, 2, 3, 5, 6).reshape(
            NV, 1024, 128
        ).astype(bf16)
        in_maps.append({"x": xdev, "wgblk": wgblk, "sel2": sel2})
    return in_maps


def kernel(edges, nodes, g_node, g_edge, W, b):
    edges = np.ascontiguousarray(edges, dtype=np.float32)
    nodes = np.ascontiguousarray(nodes, dtype=np.float32)
    g_node = np.asarray(g_node, dtype=np.float32)
    g_edge = np.asarray(g_edge, dtype=np.float32)
    W = np.asarray(W, dtype=np.float32)
    b = np.asarray(b, dtype=np.float32)

    # tiny node path on host (B*N*dn = 131K elems)
    ms = np.mean(np.square(nodes), axis=-1, keepdims=True)
    nodes_n = nodes / np.sqrt(ms + EPS) * g_node  # [B, N, 128]
    Wr, Wc, We = W[:DN], W[DN : 2 * DN], W[2 * DN :]
    row_proj = (nodes_n @ Wr).astype(np.float32)  # [B, N, 64]
    col_proj = (nodes_n @ Wc).astype(np.float32)  # [B, N, 64]

    in_maps = _make_in_maps(edges, g_edge, We)

    from concourse.bass_utils import run_bass_kernel_spmd

    nc = _get_nc()
    res = run_bass_kernel_spmd(nc, in_maps, list(range(NCORES)))

    out = np.empty((B, N, N, DE), dtype=np.float32)
    for c in range(NCORES):
        oc = res.results[c]["out"].astype(np.float32).reshape(B, IPC, N, DE)
        out[:, c * IPC : (c + 1) * IPC] = oc
    # node projections + bias added in f32 on the host
    out += row_proj[:, :, None, :] + b
    out += col_proj[:, None, :, :]
    return out


if __name__ == "__main__":
    rng = np.random.default_rng(0)
    edges = rng.standard_normal((B, N, N, DE), dtype=np.float32)
    nodes = rng.standard_normal((B, N, DN), dtype=np.float32)
    g_node = np.ones(DN, np.float32)
    g_edge = np.ones(DE, np.float32)
    W = rng.standard_normal((2 * DN + DE, DE), dtype=np.float32) / 18.0
    b = (rng.standard_normal(DE) * 0.01).astype(np.float32)
    o = kernel(edges, nodes, g_node, g_edge, W, b)
    print(o.shape, o.dtype)
